# revision 12
# baseline (speedup 1.0000x reference)
"""MoE (top-2 of 8 experts, swiglu MLP) on 8 Trainium2 NeuronCores.

Strategy: expert parallelism — core e owns expert e's weights.
 - Host: router (fp64 softmax/top-2), gather each expert's tokens,
   pre-tile weights into the layouts the PE consumes directly, cast to
   bf16 (fp32 PSUM accumulation on device keeps the error ~3.5e-3 in
   the max-over-global-max metric).
 - Device (per core, SPMD one NEFF): single chunk of C=2048 columns
   (exactly T*K/8 — perfectly balanced), all matmul N-blocks 512.
   Stage A: hT = silu(gate_w.T @ xT) * (up_w.T @ xT); stage B:
   yT = down_w @ hT. Weights stream once (no re-chunking).
 - Host: combine — scale rows by gating weight and scatter-add into
   the full output. Tokens beyond the 2048-capacity of an expert are
   computed on the host (103 token-expert pairs for this input).

Shapes: T=8192 tokens, H=2048, F=1408, E=8, K=2, C=2048.
"""

import numpy as np

T, H, E, K, F = 8192, 2048, 8, 2, 1408
C = 2048  # token capacity per expert; overflow falls back to host
N_CORES = 8
NCB = C // 512  # 512-wide column blocks

_compiled = None


def _build():
    from contextlib import ExitStack

    import concourse.mybir as mybir
    import concourse.tile as tile
    from concourse import bacc

    f32 = mybir.dt.float32
    bf16 = mybir.dt.bfloat16

    nc = bacc.Bacc("TRN2", target_bir_lowering=False, debug=False, num_devices=N_CORES)
    xt = nc.dram_tensor("xt", [H, C], bf16, kind="ExternalInput").ap()
    gu = nc.dram_tensor("gu", [2, 11, 128, 2048], bf16, kind="ExternalInput").ap()
    dw = nc.dram_tensor("dw", [16, 128, 1408], bf16, kind="ExternalInput").ap()
    yt = nc.dram_tensor("yt", [H, C], f32, kind="ExternalOutput").ap()

    with tile.TileContext(nc) as tc:
        with ExitStack() as ctx:
            pool_xt = ctx.enter_context(tc.tile_pool(name="xt", bufs=32))
            pool_gu = ctx.enter_context(tc.tile_pool(name="gu", bufs=3))
            pool_gw0 = ctx.enter_context(tc.tile_pool(name="gw0", bufs=4))
            pool_dw = ctx.enter_context(tc.tile_pool(name="dw", bufs=3))
            pool_h = ctx.enter_context(tc.tile_pool(name="h", bufs=11))
            pool_sil = ctx.enter_context(tc.tile_pool(name="sil", bufs=4))
            pool_out = ctx.enter_context(tc.tile_pool(name="out", bufs=4))
            ps = ctx.enter_context(tc.tile_pool(name="ps", bufs=8, space="PSUM"))

            # f=1,2 weights on the gpsimd (SWDGE) ring: issued at t~0, they
            # trickle in at background priority, done long before needed
            guts, uuts = {}, {}
            for f in (1, 2):
                guts[f] = pool_gu.tile([128, 2048], bf16, tag="gut", name="gut")
                uuts[f] = pool_gu.tile([128, 2048], bf16, tag="uut", name="uut")
                nc.gpsimd.dma_start(guts[f][:], gu[0, f])
                nc.gpsimd.dma_start(uuts[f][:], gu[1, f])

            # f=0 weights as 4 pieces per ring ([128,512] = h-blocks 4j..4j+3)
            # interleaved AHEAD of the xt tiles they gate, so the first
            # matmul starts ~2us after the preamble instead of ~9us
            gta = [
                pool_gw0.tile([128, 512], bf16, tag="gta", name="gta")
                for _ in range(4)
            ]
            uta = [
                pool_gw0.tile([128, 512], bf16, tag="uta", name="uta")
                for _ in range(4)
            ]

            # token activations, H on partitions. Half-tiles (cols 0:1024
            # first, then 1024:2048) alternating across the two HWDGE rings:
            # stage A's first pass consumes cols 0:1024 of each h-tile at
            # ~850ns each, matching the ~700ns DMA delivery rate, so the PE
            # never starves at startup.
            xts = [[None] * 16, [None] * 16]  # [half][hb] -> [128, 1024] tile
            for half in range(2):
                for hb in range(16):
                    xts[half][hb] = pool_xt.tile(
                        [128, C // 2], bf16, tag="xtile", name="xtile"
                    )

            def xt_dma(half, hb):
                eng = nc.sync if hb % 2 == 0 else nc.scalar
                eng.dma_start(
                    xts[half][hb][:],
                    xt[hb * 128 : (hb + 1) * 128, half * 1024 : (half + 1) * 1024],
                )

            for j in range(4):
                nc.sync.dma_start(gta[j][:], gu[0, 0][:, j * 512 : (j + 1) * 512])
                nc.scalar.dma_start(uta[j][:], gu[1, 0][:, j * 512 : (j + 1) * 512])
                xt_dma(0, 2 * j)
                xt_dma(0, 2 * j + 1)
            for hb in range(8, 16):
                xt_dma(0, hb)
            for hb in range(16):
                xt_dma(1, hb)

            # stage A: hT[f, c] = silu(gT) * uT, gT = gate_w.T @ x.T
            # g and u interleaved per h-tile in column-block pairs, so each
            # xt (half-)tile is fully consumed as soon as it lands.
            hts = []
            for f in range(11):
                if f == 0:
                    gsl = lambda h: gta[h // 4][:, (h % 4) * 128 : (h % 4 + 1) * 128]
                    usl = lambda h: uta[h // 4][:, (h % 4) * 128 : (h % 4 + 1) * 128]
                else:
                    if f in guts:
                        gut, uut = guts[f], uuts[f]
                    else:
                        gut = pool_gu.tile([128, 2048], bf16, tag="gut", name="gut")
                        uut = pool_gu.tile([128, 2048], bf16, tag="uut", name="uut")
                        nc.sync.dma_start(gut[:], gu[0, f])
                        nc.scalar.dma_start(uut[:], gu[1, f])
                    gsl = lambda h, g=gut: g[:, h * 128 : (h + 1) * 128]
                    usl = lambda h, u=uut: u[:, h * 128 : (h + 1) * 128]
                ht = pool_h.tile([128, C], bf16, tag="ht")
                hts.append(ht)
                for part in range(2):
                    cbs = (0, 1)  # column blocks within this half's tiles
                    pgs = [ps.tile([128, 512], f32, tag="ps", name="pg") for _ in cbs]
                    pus = [ps.tile([128, 512], f32, tag="ps", name="pu") for _ in cbs]
                    for h in range(16):
                        for cb, pg in zip(cbs, pgs):
                            nc.tensor.matmul(
                                pg[:],
                                gsl(h),
                                xts[part][h][:, cb * 512 : (cb + 1) * 512],
                                start=(h == 0),
                                stop=(h == 15),
                            )
                        for cb, pu in zip(cbs, pus):
                            nc.tensor.matmul(
                                pu[:],
                                usl(h),
                                xts[part][h][:, cb * 512 : (cb + 1) * 512],
                                start=(h == 0),
                                stop=(h == 15),
                            )
                    for cb, pg, pu in zip(cbs, pgs, pus):
                        gcb = 2 * part + cb
                        sil = pool_sil.tile([128, 512], f32, tag="sil")
                        nc.scalar.activation(
                            sil[:], pg[:], mybir.ActivationFunctionType.Silu
                        )
                        nc.vector.tensor_mul(
                            ht[:, gcb * 512 : (gcb + 1) * 512], sil[:], pu[:]
                        )

            # stage B: yT[h, c] = down_w @ hT  (gating applied on host)
            for hb in range(16):
                dwt = pool_dw.tile([128, 1408], bf16, tag="dwt")
                nc.sync.dma_start(dwt[:], dw[hb])
                for cb in range(NCB):
                    po = ps.tile([128, 512], f32, tag="ps", name="po")
                    for f in range(11):
                        nc.tensor.matmul(
                            po[:],
                            dwt[:, f * 128 : (f + 1) * 128],
                            hts[f][:, cb * 512 : (cb + 1) * 512],
                            start=(f == 0),
                            stop=(f == 10),
                        )
                    ot = pool_out.tile([128, 512], f32, tag="ot")
                    nc.vector.tensor_copy(ot[:], po[:])
                    nc.scalar.dma_start(
                        yt[hb * 128 : (hb + 1) * 128, cb * 512 : (cb + 1) * 512],
                        ot[:],
                    )
    nc.compile()
    return nc


def _get_compiled():
    global _compiled
    if _compiled is None:
        _compiled = _build()
    return _compiled


def _route(x, router_w):
    """fp64 router: returns per-expert (indices, gating weights)."""
    logits = x.astype(np.float64) @ router_w.astype(np.float64).T
    logits -= logits.max(axis=-1, keepdims=True)
    p = np.exp(logits)
    p /= p.sum(axis=-1, keepdims=True)
    top2 = np.argsort(-p, axis=-1)[:, :K]
    pv = np.take_along_axis(p, top2, axis=-1)
    wts = pv / (pv.sum(axis=-1, keepdims=True) + 1e-20)
    idxs, gws = [], []
    for e in range(E):
        tok, pos = np.nonzero(top2 == e)
        idxs.append(tok.astype(np.int64))
        gws.append(wts[tok, pos].astype(np.float32))
    return idxs, gws


def _tile_gu(wT):
    # gu[f_blk, k, hb*128+m] = wT[hb*128+k, f_blk*128+m]
    return (
        wT.reshape(16, 128, 11, 128)
        .transpose(2, 1, 0, 3)
        .reshape(11, 128, 2048)
        .copy()
    )


def _tile_dw(D):
    # dw[hb, k, f_blk*128+m] = D[hb*128+m, f_blk*128+k]
    return (
        D.reshape(16, 128, 11, 128).transpose(0, 3, 2, 1).reshape(16, 128, 1408).copy()
    )


def _swiglu_host(xg, gate, up, down):
    g = xg @ gate.T
    u = xg @ up.T
    h = (g / (1.0 + np.exp(-g))) * u
    return h @ down.T


def kernel(hidden_states, router_w, gate_w, up_w, down_w):
    import ml_dtypes
    from concourse import bass_utils

    bf16 = ml_dtypes.bfloat16
    x = np.ascontiguousarray(hidden_states.reshape(-1, H).astype(np.float32))
    idxs, gws = _route(x, router_w)

    in_maps = []
    spill = []  # (expert, token_indices, weights) handled on host
    for e in range(E):
        idx = idxs[e]
        if len(idx) > C:
            spill.append((e, idx[C:], gws[e][C:]))
            idx = idx[:C]
        xt = np.zeros((H, C), dtype=bf16)
        xt[:, : len(idx)] = x[idx].T.astype(bf16)
        gu = np.stack(
            [
                _tile_gu(gate_w[e].T.astype(np.float32)),
                _tile_gu(up_w[e].T.astype(np.float32)),
            ]
        ).astype(bf16)
        dw = _tile_dw(down_w[e].astype(np.float32)).astype(bf16)
        in_maps.append({"xt": xt, "gu": gu, "dw": dw})

    global _last_in_maps
    _last_in_maps = in_maps
    nc = _get_compiled()
    res = bass_utils.run_bass_kernel_spmd(
        nc, in_maps, core_ids=list(range(N_CORES))
    )

    out = np.zeros((T, H), dtype=np.float32)
    for e in range(E):
        # token indices are unique within one expert (a token's two experts
        # are distinct), so fancy-index += is an exact scatter-add
        idx = idxs[e][:C]
        w = gws[e][:C]
        y = res.results[e]["yt"][:, : len(idx)].T
        out[idx] += w[:, None] * y
    for e, idx, w in spill:
        y = _swiglu_host(x[idx], gate_w[e], up_w[e], down_w[e]).astype(np.float32)
        out[idx] += w[:, None] * y
    return out.reshape(hidden_states.shape).astype(np.float32)


# revision 14
# speedup vs baseline: 1.0019x; 1.0019x over previous
"""MoE (top-2 of 8 experts, swiglu MLP) on 8 Trainium2 NeuronCores.

Strategy: expert parallelism — core e owns expert e's weights.
 - Host: router (fp64 softmax/top-2), gather each expert's tokens,
   pre-tile weights into the layouts the PE consumes directly, cast to
   bf16 (fp32 PSUM accumulation on device keeps the error ~3.5e-3 in
   the max-over-global-max metric).
 - Device (per core, SPMD one NEFF): single chunk of C=2048 columns
   (exactly T*K/8 — perfectly balanced), all matmul N-blocks 512.
   Stage A: hT = silu(gate_w.T @ xT) * (up_w.T @ xT); stage B:
   yT = down_w @ hT. Weights stream once (no re-chunking).
 - Host: combine — scale rows by gating weight and scatter-add into
   the full output. Tokens beyond the 2048-capacity of an expert are
   computed on the host (103 token-expert pairs for this input).

Shapes: T=8192 tokens, H=2048, F=1408, E=8, K=2, C=2048.
"""

import numpy as np

T, H, E, K, F = 8192, 2048, 8, 2, 1408
C = 2048  # token capacity per expert; overflow falls back to host
N_CORES = 8
NCB = C // 512  # 512-wide column blocks

_compiled = None


def _build():
    from contextlib import ExitStack

    import concourse.mybir as mybir
    import concourse.tile as tile
    from concourse import bacc

    f32 = mybir.dt.float32
    bf16 = mybir.dt.bfloat16

    nc = bacc.Bacc("TRN2", target_bir_lowering=False, debug=False, num_devices=N_CORES)
    xt = nc.dram_tensor("xt", [H, C], bf16, kind="ExternalInput").ap()
    gu = nc.dram_tensor("gu", [2, 11, 128, 2048], bf16, kind="ExternalInput").ap()
    dw = nc.dram_tensor("dw", [16, 128, 1408], bf16, kind="ExternalInput").ap()
    yt = nc.dram_tensor("yt", [H, C], f32, kind="ExternalOutput").ap()

    with tile.TileContext(nc) as tc:
        with ExitStack() as ctx:
            pool_xt = ctx.enter_context(tc.tile_pool(name="xt", bufs=32))
            pool_gu = ctx.enter_context(tc.tile_pool(name="gu", bufs=3))
            pool_gw0 = ctx.enter_context(tc.tile_pool(name="gw0", bufs=4))
            pool_dw = ctx.enter_context(tc.tile_pool(name="dw", bufs=3))
            pool_h = ctx.enter_context(tc.tile_pool(name="h", bufs=11))
            pool_sil = ctx.enter_context(tc.tile_pool(name="sil", bufs=4))
            pool_out = ctx.enter_context(tc.tile_pool(name="out", bufs=4))
            ps = ctx.enter_context(tc.tile_pool(name="ps", bufs=8, space="PSUM"))

            # f=0 weights as 4 pieces per ring ([128,512] = h-blocks 4j..4j+3)
            # interleaved AHEAD of the xt tiles they gate, so the first
            # matmul starts ~2us after the preamble instead of ~9us
            gta = [
                pool_gw0.tile([128, 512], bf16, tag="gta", name="gta")
                for _ in range(4)
            ]
            uta = [
                pool_gw0.tile([128, 512], bf16, tag="uta", name="uta")
                for _ in range(4)
            ]

            # token activations, H on partitions. Half-tiles (cols 0:1024
            # first, then 1024:2048) alternating across the two HWDGE rings:
            # stage A's first pass consumes cols 0:1024 of each h-tile at
            # ~850ns each, matching the ~700ns DMA delivery rate, so the PE
            # never starves at startup.
            xts = [[None] * 16, [None] * 16]  # [half][hb] -> [128, 1024] tile
            for half in range(2):
                for hb in range(16):
                    xts[half][hb] = pool_xt.tile(
                        [128, C // 2], bf16, tag="xtile", name="xtile"
                    )

            def xt_dma(half, hb):
                eng = nc.sync if hb % 2 == 0 else nc.scalar
                eng.dma_start(
                    xts[half][hb][:],
                    xt[hb * 128 : (hb + 1) * 128, half * 1024 : (half + 1) * 1024],
                )

            for j in range(4):
                nc.sync.dma_start(gta[j][:], gu[0, 0][:, j * 512 : (j + 1) * 512])
                nc.scalar.dma_start(uta[j][:], gu[1, 0][:, j * 512 : (j + 1) * 512])
                xt_dma(0, 2 * j)
                xt_dma(0, 2 * j + 1)
            for hb in range(8, 16):
                xt_dma(0, hb)
            for hb in range(16):
                xt_dma(1, hb)
            # f=1,2 weights ride the same rings right behind the xt tiles:
            # they land just before stage A reaches f=1 / f=2 and, crucially,
            # do not steal HBM bandwidth from the startup-critical xt loads
            guts, uuts = {}, {}
            for f in (1, 2):
                guts[f] = pool_gu.tile([128, 2048], bf16, tag="gut", name="gut")
                uuts[f] = pool_gu.tile([128, 2048], bf16, tag="uut", name="uut")
                nc.sync.dma_start(guts[f][:], gu[0, f])
                nc.scalar.dma_start(uuts[f][:], gu[1, f])

            # stage A: hT[f, c] = silu(gT) * uT, gT = gate_w.T @ x.T
            # g and u interleaved per h-tile in column-block pairs, so each
            # xt (half-)tile is fully consumed as soon as it lands.
            hts = []
            for f in range(11):
                if f == 0:
                    gsl = lambda h: gta[h // 4][:, (h % 4) * 128 : (h % 4 + 1) * 128]
                    usl = lambda h: uta[h // 4][:, (h % 4) * 128 : (h % 4 + 1) * 128]
                else:
                    if f in guts:
                        gut, uut = guts[f], uuts[f]
                    else:
                        gut = pool_gu.tile([128, 2048], bf16, tag="gut", name="gut")
                        uut = pool_gu.tile([128, 2048], bf16, tag="uut", name="uut")
                        nc.sync.dma_start(gut[:], gu[0, f])
                        nc.scalar.dma_start(uut[:], gu[1, f])
                    gsl = lambda h, g=gut: g[:, h * 128 : (h + 1) * 128]
                    usl = lambda h, u=uut: u[:, h * 128 : (h + 1) * 128]
                ht = pool_h.tile([128, C], bf16, tag="ht")
                hts.append(ht)
                for part in range(2):
                    cbs = (0, 1)  # column blocks within this half's tiles
                    pgs = [ps.tile([128, 512], f32, tag="ps", name="pg") for _ in cbs]
                    pus = [ps.tile([128, 512], f32, tag="ps", name="pu") for _ in cbs]
                    for h in range(16):
                        for cb, pg in zip(cbs, pgs):
                            nc.tensor.matmul(
                                pg[:],
                                gsl(h),
                                xts[part][h][:, cb * 512 : (cb + 1) * 512],
                                start=(h == 0),
                                stop=(h == 15),
                            )
                        for cb, pu in zip(cbs, pus):
                            nc.tensor.matmul(
                                pu[:],
                                usl(h),
                                xts[part][h][:, cb * 512 : (cb + 1) * 512],
                                start=(h == 0),
                                stop=(h == 15),
                            )
                    for cb, pg, pu in zip(cbs, pgs, pus):
                        gcb = 2 * part + cb
                        sil = pool_sil.tile([128, 512], f32, tag="sil")
                        nc.scalar.activation(
                            sil[:], pg[:], mybir.ActivationFunctionType.Silu
                        )
                        nc.vector.tensor_mul(
                            ht[:, gcb * 512 : (gcb + 1) * 512], sil[:], pu[:]
                        )

            # stage B: yT[h, c] = down_w @ hT  (gating applied on host)
            for hb in range(16):
                dwt = pool_dw.tile([128, 1408], bf16, tag="dwt")
                nc.sync.dma_start(dwt[:], dw[hb])
                for cb in range(NCB):
                    po = ps.tile([128, 512], f32, tag="ps", name="po")
                    for f in range(11):
                        nc.tensor.matmul(
                            po[:],
                            dwt[:, f * 128 : (f + 1) * 128],
                            hts[f][:, cb * 512 : (cb + 1) * 512],
                            start=(f == 0),
                            stop=(f == 10),
                        )
                    ot = pool_out.tile([128, 512], f32, tag="ot")
                    nc.vector.tensor_copy(ot[:], po[:])
                    nc.scalar.dma_start(
                        yt[hb * 128 : (hb + 1) * 128, cb * 512 : (cb + 1) * 512],
                        ot[:],
                    )
    nc.compile()
    return nc


def _get_compiled():
    global _compiled
    if _compiled is None:
        _compiled = _build()
    return _compiled


def _route(x, router_w):
    """fp64 router: returns per-expert (indices, gating weights)."""
    logits = x.astype(np.float64) @ router_w.astype(np.float64).T
    logits -= logits.max(axis=-1, keepdims=True)
    p = np.exp(logits)
    p /= p.sum(axis=-1, keepdims=True)
    top2 = np.argsort(-p, axis=-1)[:, :K]
    pv = np.take_along_axis(p, top2, axis=-1)
    wts = pv / (pv.sum(axis=-1, keepdims=True) + 1e-20)
    idxs, gws = [], []
    for e in range(E):
        tok, pos = np.nonzero(top2 == e)
        idxs.append(tok.astype(np.int64))
        gws.append(wts[tok, pos].astype(np.float32))
    return idxs, gws


def _tile_gu(wT):
    # gu[f_blk, k, hb*128+m] = wT[hb*128+k, f_blk*128+m]
    return (
        wT.reshape(16, 128, 11, 128)
        .transpose(2, 1, 0, 3)
        .reshape(11, 128, 2048)
        .copy()
    )


def _tile_dw(D):
    # dw[hb, k, f_blk*128+m] = D[hb*128+m, f_blk*128+k]
    return (
        D.reshape(16, 128, 11, 128).transpose(0, 3, 2, 1).reshape(16, 128, 1408).copy()
    )


def _swiglu_host(xg, gate, up, down):
    g = xg @ gate.T
    u = xg @ up.T
    h = (g / (1.0 + np.exp(-g))) * u
    return h @ down.T


def kernel(hidden_states, router_w, gate_w, up_w, down_w):
    import ml_dtypes
    from concourse import bass_utils

    bf16 = ml_dtypes.bfloat16
    x = np.ascontiguousarray(hidden_states.reshape(-1, H).astype(np.float32))
    idxs, gws = _route(x, router_w)

    in_maps = []
    spill = []  # (expert, token_indices, weights) handled on host
    for e in range(E):
        idx = idxs[e]
        if len(idx) > C:
            spill.append((e, idx[C:], gws[e][C:]))
            idx = idx[:C]
        xt = np.zeros((H, C), dtype=bf16)
        xt[:, : len(idx)] = x[idx].T.astype(bf16)
        gu = np.stack(
            [
                _tile_gu(gate_w[e].T.astype(np.float32)),
                _tile_gu(up_w[e].T.astype(np.float32)),
            ]
        ).astype(bf16)
        dw = _tile_dw(down_w[e].astype(np.float32)).astype(bf16)
        in_maps.append({"xt": xt, "gu": gu, "dw": dw})

    global _last_in_maps
    _last_in_maps = in_maps
    nc = _get_compiled()
    res = bass_utils.run_bass_kernel_spmd(
        nc, in_maps, core_ids=list(range(N_CORES))
    )

    out = np.zeros((T, H), dtype=np.float32)
    for e in range(E):
        # token indices are unique within one expert (a token's two experts
        # are distinct), so fancy-index += is an exact scatter-add
        idx = idxs[e][:C]
        w = gws[e][:C]
        y = res.results[e]["yt"][:, : len(idx)].T
        out[idx] += w[:, None] * y
    for e, idx, w in spill:
        y = _swiglu_host(x[idx], gate_w[e], up_w[e], down_w[e]).astype(np.float32)
        out[idx] += w[:, None] * y
    return out.reshape(hidden_states.shape).astype(np.float32)
